# revision 6
# baseline (speedup 1.0000x reference)
"""AttentionGate kernel for Trainium2 (8 NeuronCores, data-parallel over batch).

Per core (batch element b):
  qp = q @ Wq + bq ; kp = k @ Wk + bk ; vp = v @ Wv + bv      [S, D]
  scores^T = kp @ qp^T / sqrt(D)                               [S_k, S_q]
  attn = exp(scores) (no max subtraction; |scores| <~ 1.3)
  out = (attn @ vp) / rowsum(attn)                             [S, D]

Layouts (SBUF):
  qT/kT/vT  [d, s] bf16  (feature-major, via DVE cast + xbar DMA transpose)
  qpT/kpT   [d, s] bf16  (projection outputs, feature-major)
  vp        [s, d] bf16  (natural; bias added via K=1 rank-1 matmul)
  expT      [k, q] bf16  (scores pre-transposed so attn@vp needs no transpose;
                          rowsum via N=1 matmuls on the already-loaded lhsT)
"""

import numpy as np

B, S, D = 8, 2048, 512
P = 128
HT = D // P            # 4 feature tiles
ST = S // P            # 16 sequence tiles
QC = 512               # q-chunk (moving free dim) for scores
NQC = S // QC          # 4 q chunks
SCALE = 1.0 / np.sqrt(np.float32(D))

_NC = None


def build_nc():
    import concourse.mybir as mybir
    import concourse.tile as tile
    from concourse import bacc
    from concourse._compat import get_trn_type

    f32 = mybir.dt.float32
    bf16 = mybir.dt.bfloat16

    nc = bacc.Bacc(get_trn_type() or "TRN2", target_bir_lowering=False, debug=False)
    q_h = nc.dram_tensor("q", [S, D], f32, kind="ExternalInput")
    k_h = nc.dram_tensor("k", [S, D], f32, kind="ExternalInput")
    v_h = nc.dram_tensor("v", [S, D], f32, kind="ExternalInput")
    Wq_h = nc.dram_tensor("Wq", [D, D], f32, kind="ExternalInput")
    bq_h = nc.dram_tensor("bq", [D], f32, kind="ExternalInput")
    Wk_h = nc.dram_tensor("Wk", [D, D], f32, kind="ExternalInput")
    bk_h = nc.dram_tensor("bk", [D], f32, kind="ExternalInput")
    Wv_h = nc.dram_tensor("Wv", [D, D], f32, kind="ExternalInput")
    bv_h = nc.dram_tensor("bv", [D], f32, kind="ExternalInput")
    out_h = nc.dram_tensor("out", [S, D], f32, kind="ExternalOutput")

    with tile.TileContext(nc) as tc:
        with (
            tc.tile_pool(name="consts", bufs=1) as consts,
            tc.tile_pool(name="ld", bufs=4) as ld,
            tc.tile_pool(name="nat", bufs=6) as nat,
            tc.tile_pool(name="big", bufs=1) as big,
            tc.tile_pool(name="expp", bufs=2) as expp,
            tc.tile_pool(name="outp", bufs=3) as outp,
            tc.tile_pool(name="small", bufs=4) as small,
            tc.tile_pool(name="ps_mm", bufs=3, space="PSUM") as ps_mm,
            tc.tile_pool(name="ps_out", bufs=2, space="PSUM") as ps_out,
            tc.tile_pool(name="ps_rs", bufs=2, space="PSUM") as ps_rs,
        ):
            # ---- constants: weights (bf16, [h, d] natural), biases, ones ----
            w_sbs = []
            for i, W_h in enumerate((Wq_h, Wk_h, Wv_h)):
                w_sb = consts.tile([P, HT, D], bf16, tag=f"w{i}")
                for hi in range(HT):
                    wf = ld.tile([P, D], f32, tag="ld")
                    nc.sync.dma_start(out=wf, in_=W_h[hi * P:(hi + 1) * P, :])
                    nc.vector.tensor_copy(out=w_sb[:, hi, :], in_=wf)
                w_sbs.append(w_sb)
            wq, wk, wv = w_sbs

            bq_sb = consts.tile([P, HT], f32, tag="bq")
            nc.sync.dma_start(out=bq_sb, in_=bq_h[:].rearrange("(di p) -> p di", p=P))
            bk_sb = consts.tile([P, HT], f32, tag="bk")
            nc.sync.dma_start(out=bk_sb, in_=bk_h[:].rearrange("(di p) -> p di", p=P))

            bvf = ld.tile([P, D], f32, tag="ld")
            nc.sync.dma_start(out=bvf[:1, :], in_=bv_h[:].rearrange("(a d) -> a d", a=1))
            bv_bf = consts.tile([1, D], bf16, tag="bv")
            nc.vector.tensor_copy(out=bv_bf, in_=bvf[:1, :])

            ones_bf = consts.tile([P, P], bf16, tag="ones")
            nc.vector.memset(ones_bf, 1.0)

            # ---- inputs -> feature-major bf16 (cast on DVE, transpose on xbar) ----
            def load_T(x_h, tag):
                xT = big.tile([P, HT, S], bf16, tag=tag)
                for st in range(ST):
                    xf = ld.tile([P, D], f32, tag="ld")
                    nc.sync.dma_start(out=xf, in_=x_h[st * P:(st + 1) * P, :])
                    xb = nat.tile([P, D], bf16, tag="nat")
                    nc.vector.tensor_copy(out=xb, in_=xf)
                    for hi in range(HT):
                        nc.sync.dma_start_transpose(
                            out=xT[:, hi, st * P:(st + 1) * P],
                            in_=xb[:, hi * P:(hi + 1) * P],
                        )
                return xT

            kT = load_T(k_h, "kT")
            vT = load_T(v_h, "vT")
            qT = load_T(q_h, "qT")

            # ---- projections ----
            # kpT/qpT: [d, s] bf16, bias per-partition
            def project_T(xT, w_sb, b_sb, tag):
                xpT = big.tile([P, HT, S], bf16, tag=tag)
                for di in range(HT):
                    for sc in range(NQC):
                        ps = ps_mm.tile([P, QC], f32, tag="ps_mm")
                        for hi in range(HT):
                            nc.tensor.matmul(
                                ps,
                                w_sb[:, hi, di * P:(di + 1) * P],
                                xT[:, hi, sc * QC:(sc + 1) * QC],
                                start=(hi == 0),
                                stop=(hi == HT - 1),
                            )
                        nc.vector.tensor_scalar_add(
                            out=xpT[:, di, sc * QC:(sc + 1) * QC],
                            in0=ps,
                            scalar1=b_sb[:, di:di + 1],
                        )
                return xpT

            kpT = project_T(kT, wk, bk_sb, "kpT")
            qpT = project_T(qT, wq, bq_sb, "qpT")

            # vp: [s, d] bf16 natural; bias via K=1 rank-1 matmul into PSUM
            vp = big.tile([P, ST, D], bf16, tag="vp")
            for st in range(ST):
                ps = ps_mm.tile([P, D], f32, tag="ps_mm")
                for hi in range(HT):
                    nc.tensor.matmul(
                        ps,
                        vT[:, hi, st * P:(st + 1) * P],
                        wv[:, hi, :],
                        start=(hi == 0),
                        stop=False,
                    )
                nc.tensor.matmul(ps, ones_bf[:1, :], bv_bf, start=False, stop=True)
                nc.vector.tensor_copy(out=vp[:, st, :], in_=ps)

            # ---- scores^T -> exp -> attn@vp, software-pipelined over q chunks ----
            def scores_chunk(qc, ex):
                for kt in range(ST):
                    ps = ps_mm.tile([P, QC], f32, tag="ps_mm")
                    for di in range(HT):
                        nc.tensor.matmul(
                            ps,
                            kpT[:, di, kt * P:(kt + 1) * P],
                            qpT[:, di, qc * QC:(qc + 1) * QC],
                            start=(di == 0),
                            stop=(di == HT - 1),
                        )
                    nc.scalar.activation(
                        out=ex[:, kt, :],
                        in_=ps,
                        func=mybir.ActivationFunctionType.Exp,
                        scale=float(SCALE),
                    )

            def attn_chunk(qc, ex):
                for qi in range(QC // P):
                    ps_o = ps_out.tile([P, D], f32, tag="ps_out")
                    ps_r = ps_rs.tile([P, 1], f32, tag="ps_rs")
                    for kt in range(ST):
                        lt = ex[:, kt, qi * P:(qi + 1) * P]
                        nc.tensor.matmul(
                            ps_o, lt, vp[:, kt, :],
                            start=(kt == 0), stop=(kt == ST - 1),
                        )
                        nc.tensor.matmul(
                            ps_r, lt, ones_bf[:, :1],
                            start=(kt == 0), stop=(kt == ST - 1),
                        )
                    rc = small.tile([P, 1], f32, tag="rc")
                    nc.vector.reciprocal(rc, ps_r)
                    ob = outp.tile([P, D], f32, tag="ob")
                    nc.vector.tensor_scalar_mul(ob, ps_o, rc)
                    qt = qc * (QC // P) + qi
                    nc.sync.dma_start(
                        out=out_h[qt * P:(qt + 1) * P, :], in_=ob
                    )

            exs = []
            for qc in range(NQC):
                ex = expp.tile([P, ST, QC], bf16, tag="ex")
                scores_chunk(qc, ex)
                exs.append(ex)
                if qc >= 1:
                    attn_chunk(qc - 1, exs[qc - 1])
            attn_chunk(NQC - 1, exs[NQC - 1])

    nc.compile()
    return nc


def _get_nc():
    global _NC
    if _NC is None:
        _NC = build_nc()
    return _NC


def build_in_maps(q, k, v, Wq, bq, Wk, bk, Wv, bv):
    in_maps = []
    for b in range(B):
        in_maps.append({
            "q": np.ascontiguousarray(q[b], dtype=np.float32),
            "k": np.ascontiguousarray(k[b], dtype=np.float32),
            "v": np.ascontiguousarray(v[b], dtype=np.float32),
            "Wq": np.ascontiguousarray(Wq, dtype=np.float32),
            "bq": np.ascontiguousarray(bq, dtype=np.float32),
            "Wk": np.ascontiguousarray(Wk, dtype=np.float32),
            "bk": np.ascontiguousarray(bk, dtype=np.float32),
            "Wv": np.ascontiguousarray(Wv, dtype=np.float32),
            "bv": np.ascontiguousarray(bv, dtype=np.float32),
        })
    return in_maps


def kernel(q, k, v, Wq, bq, Wk, bk, Wv, bv):
    from concourse.bass_utils import run_bass_kernel_spmd

    nc = _get_nc()
    in_maps = build_in_maps(q, k, v, Wq, bq, Wk, bk, Wv, bv)
    res = run_bass_kernel_spmd(nc, in_maps, core_ids=list(range(B)))
    return np.stack([r["out"] for r in res.results], axis=0)


# revision 38
# speedup vs baseline: 1.3161x; 1.3161x over previous
"""AttentionGate kernel for Trainium2 (8 NeuronCores, data-parallel over batch).

Per core (batch element b):
  qp = q @ Wq + bq ; kp = k @ Wk + bk ; vp = v @ Wv + bv      [S, D]
  scores^T = kp @ qp^T / sqrt(D)                               [S_k, S_q]
  attn = exp(scores) (no max subtraction; |scores| <~ 1.3)
  out = (attn @ vp) / rowsum(attn)                             [S, D]

Layouts (SBUF):
  qT/kT/vT  [d, s] bf16  (feature-major, via DVE cast + xbar DMA transpose)
  qpT/kpT   [d, s] bf16  (projection outputs, feature-major)
  vp        [s, d] bf16  (natural; bias added on DVE via broadcast row)
  expT      [k, q] bf16  (scores pre-transposed so attn@vp needs no transpose;
                          rowsum via N=1 matmuls on the already-loaded lhsT)

Scheduling: the DMA wire is one serial resource and copy<->transpose
transitions cross-serialize, so the exact wire order is pinned with a chain
of ordering deps on one HWDGE engine. Input chunk deliveries of k/q/v are
interleaved and PE consumes them as a wavefront: scores block SB(qc, kc)
needs only qpT chunk qc and kpT chunk kc.
"""

import numpy as np

B, S, D = 8, 2048, 512
P = 128
HT = D // P            # 4 feature tiles
ST = S // P            # 16 sequence tiles
QC = 512               # q-chunk (moving free dim) for scores
NQC = S // QC          # 4 q chunks
SCALE = 1.0 / np.sqrt(np.float32(D))

_NC = None


def build_nc(repeat=1):
    import concourse.bass as bass
    import concourse.mybir as mybir
    import concourse.tile as tile
    from concourse import bacc
    from concourse._compat import get_trn_type
    from concourse.tile_rust import add_dep_helper

    f32 = mybir.dt.float32
    bf16 = mybir.dt.bfloat16

    nc = bacc.Bacc(get_trn_type() or "TRN2", target_bir_lowering=False, debug=False)
    q_h = nc.dram_tensor("q", [S, D], f32, kind="ExternalInput")
    k_h = nc.dram_tensor("k", [S, D], f32, kind="ExternalInput")
    v_h = nc.dram_tensor("v", [S, D], f32, kind="ExternalInput")
    Wq_h = nc.dram_tensor("Wq", [D, D], f32, kind="ExternalInput")
    bq_h = nc.dram_tensor("bq", [D], f32, kind="ExternalInput")
    Wk_h = nc.dram_tensor("Wk", [D, D], f32, kind="ExternalInput")
    bk_h = nc.dram_tensor("bk", [D], f32, kind="ExternalInput")
    Wv_h = nc.dram_tensor("Wv", [D, D], f32, kind="ExternalInput")
    bv_h = nc.dram_tensor("bv", [D], f32, kind="ExternalInput")
    out_h = nc.dram_tensor("out", [S, D], f32, kind="ExternalOutput")

    with tile.TileContext(nc) as tc:
        with (
            tc.tile_pool(name="consts", bufs=1) as consts,
            tc.tile_pool(name="ld", bufs=2) as ld,
            tc.tile_pool(name="nat", bufs=3) as nat,
            tc.tile_pool(name="big", bufs=1) as big,
            tc.tile_pool(name="expp", bufs=3) as expp,
            tc.tile_pool(name="outp", bufs=3) as outp,
            tc.tile_pool(name="small", bufs=4) as small,
            tc.tile_pool(name="ps_mm", bufs=3, space="PSUM") as ps_mm,
            tc.tile_pool(name="ps_out", bufs=2, space="PSUM") as ps_out,
            tc.tile_pool(name="ps_rs", bufs=2, space="PSUM") as ps_rs,
        ):
            chain = [None]

            def chained(ins):
                if chain[0] is not None:
                    add_dep_helper(ins.ins, chain[0].ins, sync=False,
                                   reason="wire order")
                chain[0] = ins
                return ins

            def load_W(W_h, i):
                w_sb = consts.tile([P, HT, D], bf16, tag=f"w{i}")
                wf = ld.tile([P, HT, D], f32, tag="ld")
                chained(nc.sync.dma_start(
                    out=wf, in_=W_h[:].rearrange("(c p) j -> p c j", p=P)))
                nc.vector.tensor_copy(out=w_sb, in_=wf)
                return w_sb

            # one input chunk: 1 MB load + DVE cast + 4x 4-in-1 xbar
            # transposes (out [p, hi, j] holds logical row hi*128+p)
            def load_chunk(x_h, xT, sc):
                xf = ld.tile([P, HT, D], f32, tag="ld")
                chained(nc.sync.dma_start(
                    out=xf,
                    in_=x_h[sc * QC:(sc + 1) * QC, :].rearrange(
                        "(c p) j -> p c j", p=P),
                ))
                xb = nat.tile([P, HT, D], bf16, tag="nat")
                for i in range(HT):
                    st = sc * HT + i
                    nc.vector.tensor_copy(out=xb[:, i, :], in_=xf[:, i, :])
                    chained(nc.sync.dma_start_transpose(
                        out=xT[:, :, st * P:(st + 1) * P], in_=xb[:, i, :]))

            for _rep in range(repeat):
                # small constants, chained at the wire head (HWDGE issue is
                # cheaper than Pool and they stay clear of the transposes)
                bq_sb = consts.tile([P, HT], f32, tag="bq")
                chained(nc.sync.dma_start(
                    out=bq_sb, in_=bq_h[:].rearrange("(di p) -> p di", p=P)))
                bk_sb = consts.tile([P, HT], f32, tag="bk")
                chained(nc.sync.dma_start(
                    out=bk_sb, in_=bk_h[:].rearrange("(di p) -> p di", p=P)))
                bv_bcast = consts.tile([P, D], f32, tag="bv")
                chained(nc.sync.dma_start(
                    out=bv_bcast,
                    in_=bass.AP(tensor=bv_h[:].tensor, offset=0,
                                ap=[[0, P], [1, D]]),
                ))
                ones_bf = consts.tile([P, P], bf16, tag="ones")
                nc.vector.memset(ones_bf, 1.0)

                kT = big.tile([P, HT, S], bf16, tag="kT")
                qT = big.tile([P, HT, S], bf16, tag="qT")
                vT = big.tile([P, HT, S], bf16, tag="vT")
                kpT = big.tile([P, HT, S], bf16, tag="kpT")
                qpT = big.tile([P, HT, S], bf16, tag="qpT")
                vp = big.tile([P, ST, D], bf16, tag="vp")

                # wire order: wk k0 k1 wq q0 k2 k3 q1 wv v0 v1 q2 v2 v3 q3
                wk = load_W(Wk_h, 1)
                load_chunk(k_h, kT, 0)
                load_chunk(k_h, kT, 1)
                wq = load_W(Wq_h, 0)
                load_chunk(q_h, qT, 0)
                load_chunk(k_h, kT, 2)
                load_chunk(k_h, kT, 3)
                load_chunk(q_h, qT, 1)
                wv = load_W(Wv_h, 2)
                load_chunk(v_h, vT, 0)
                load_chunk(v_h, vT, 1)
                load_chunk(q_h, qT, 2)
                load_chunk(v_h, vT, 2)
                load_chunk(v_h, vT, 3)
                load_chunk(q_h, qT, 3)

                def project_chunk(xT, w_sb, b_sb, xpT, sc):
                    for di in range(HT):
                        ps = ps_mm.tile([P, QC], f32, tag="ps_mm")
                        for hi in range(HT):
                            nc.tensor.matmul(
                                ps,
                                w_sb[:, hi, di * P:(di + 1) * P],
                                xT[:, hi, sc * QC:(sc + 1) * QC],
                                start=(hi == 0),
                                stop=(hi == HT - 1),
                            )
                        nc.vector.tensor_scalar_add(
                            out=xpT[:, di, sc * QC:(sc + 1) * QC],
                            in0=ps,
                            scalar1=b_sb[:, di:di + 1],
                        )

                def vp_chunk(sc):
                    for st in range(sc * HT, (sc + 1) * HT):
                        ps = ps_mm.tile([P, D], f32, tag="ps_mm")
                        for hi in range(HT):
                            nc.tensor.matmul(
                                ps,
                                vT[:, hi, st * P:(st + 1) * P],
                                wv[:, hi, :],
                                start=(hi == 0),
                                stop=(hi == HT - 1),
                            )
                        nc.vector.tensor_add(vp[:, st, :], ps, bv_bcast)

                # scores block: kt tiles kc*4..kc*4+3 of q-chunk qc, with exp
                def scores_block(qc, kc, ex):
                    for kt in range(kc * HT, (kc + 1) * HT):
                        ps = ps_mm.tile([P, QC], f32, tag="ps_mm")
                        for di in range(HT):
                            nc.tensor.matmul(
                                ps,
                                kpT[:, di, kt * P:(kt + 1) * P],
                                qpT[:, di, qc * QC:(qc + 1) * QC],
                                start=(di == 0),
                                stop=(di == HT - 1),
                            )
                        nc.scalar.activation(
                            out=ex[:, kt, :],
                            in_=ps,
                            func=mybir.ActivationFunctionType.Exp,
                            scale=float(SCALE),
                        )

                def attn_chunk(qc, ex):
                    for qi in range(QC // P):
                        ps_o = ps_out.tile([P, D], f32, tag="ps_out")
                        ps_r = ps_rs.tile([P, 1], f32, tag="ps_rs")
                        for kt in range(ST):
                            lt = ex[:, kt, qi * P:(qi + 1) * P]
                            nc.tensor.matmul(
                                ps_o, lt, vp[:, kt, :],
                                start=(kt == 0), stop=(kt == ST - 1),
                            )
                            nc.tensor.matmul(
                                ps_r, lt, ones_bf[:, :1],
                                start=(kt == 0), stop=(kt == ST - 1),
                            )
                        rc = small.tile([P, 1], f32, tag="rc")
                        nc.vector.reciprocal(rc, ps_r)
                        ob = outp.tile([P, D], f32, tag="ob")
                        nc.vector.tensor_scalar_mul(ob, ps_o, rc)
                        qt = qc * (QC // P) + qi
                        nc.gpsimd.dma_start(
                            out=out_h[qt * P:(qt + 1) * P, :], in_=ob)

                # PE wavefront
                exs = []
                project_chunk(kT, wk, bk_sb, kpT, 0)
                project_chunk(kT, wk, bk_sb, kpT, 1)
                project_chunk(qT, wq, bq_sb, qpT, 0)
                ex0 = expp.tile([P, ST, QC], bf16, tag="ex")
                exs.append(ex0)
                scores_block(0, 0, ex0)
                scores_block(0, 1, ex0)
                project_chunk(kT, wk, bk_sb, kpT, 2)
                scores_block(0, 2, ex0)
                project_chunk(kT, wk, bk_sb, kpT, 3)
                scores_block(0, 3, ex0)
                ex1 = expp.tile([P, ST, QC], bf16, tag="ex")
                exs.append(ex1)
                project_chunk(qT, wq, bq_sb, qpT, 1)
                for kc in range(NQC):
                    scores_block(1, kc, ex1)
                for sc in range(NQC):
                    vp_chunk(sc)
                attn_chunk(0, exs[0])
                ex2 = expp.tile([P, ST, QC], bf16, tag="ex")
                exs.append(ex2)
                project_chunk(qT, wq, bq_sb, qpT, 2)
                for kc in range(NQC):
                    scores_block(2, kc, ex2)
                attn_chunk(1, exs[1])
                ex3 = expp.tile([P, ST, QC], bf16, tag="ex")
                exs.append(ex3)
                project_chunk(qT, wq, bq_sb, qpT, 3)
                for kc in range(NQC):
                    scores_block(3, kc, ex3)
                attn_chunk(2, exs[2])
                attn_chunk(3, exs[3])

    nc.compile()
    return nc


def _get_nc():
    global _NC
    if _NC is None:
        _NC = build_nc()
    return _NC


def build_in_maps(q, k, v, Wq, bq, Wk, bk, Wv, bv):
    in_maps = []
    for b in range(B):
        in_maps.append({
            "q": np.ascontiguousarray(q[b], dtype=np.float32),
            "k": np.ascontiguousarray(k[b], dtype=np.float32),
            "v": np.ascontiguousarray(v[b], dtype=np.float32),
            "Wq": np.ascontiguousarray(Wq, dtype=np.float32),
            "bq": np.ascontiguousarray(bq, dtype=np.float32),
            "Wk": np.ascontiguousarray(Wk, dtype=np.float32),
            "bk": np.ascontiguousarray(bk, dtype=np.float32),
            "Wv": np.ascontiguousarray(Wv, dtype=np.float32),
            "bv": np.ascontiguousarray(bv, dtype=np.float32),
        })
    return in_maps


def kernel(q, k, v, Wq, bq, Wk, bk, Wv, bv):
    from concourse.bass_utils import run_bass_kernel_spmd

    nc = _get_nc()
    in_maps = build_in_maps(q, k, v, Wq, bq, Wk, bk, Wv, bv)
    res = run_bass_kernel_spmd(nc, in_maps, core_ids=list(range(B)))
    return np.stack([r["out"] for r in res.results], axis=0)


# revision 43
# speedup vs baseline: 1.6547x; 1.2573x over previous
"""AttentionGate kernel for Trainium2 (8 NeuronCores, data-parallel over batch).

Per core (batch element b):
  qp = q @ Wq + bq ; kp = k @ Wk + bk ; vp = v @ Wv + bv      [S, D]
  scores^T = kp @ qp^T / sqrt(D)                               [S_k, S_q]
  attn = exp(scores) (no max subtraction; |scores| <~ 1.3)
  out = (attn @ vp) / rowsum(attn)                             [S, D]

Layouts (SBUF):
  qT/kT/vT  [d, s] bf16  (feature-major, via DVE cast + xbar DMA transpose)
  qpT/kpT   [d, s] bf16  (projection outputs, feature-major)
  vp        [s, d] bf16  (natural; bias added on DVE via broadcast row)
  expT      [k, q] bf16  (scores pre-transposed so attn@vp needs no transpose;
                          rowsum via N=1 matmuls on the already-loaded lhsT)

Scheduling: the DMA wire is one serial resource and copy<->transpose
transitions cross-serialize, so the exact wire order is pinned with a chain
of ordering deps on one HWDGE engine. Input chunk deliveries of k/q/v are
interleaved and PE consumes them as a wavefront: scores block SB(qc, kc)
needs only qpT chunk qc and kpT chunk kc.
"""

import numpy as np

B, S, D = 8, 2048, 512
P = 128
HT = D // P            # 4 feature tiles
ST = S // P            # 16 sequence tiles
QC = 512               # q-chunk (moving free dim) for scores
NQC = S // QC          # 4 q chunks
SCALE = 1.0 / np.sqrt(np.float32(D))

_NC = None


def build_nc(repeat=1):
    import concourse.bass as bass
    import concourse.mybir as mybir
    import concourse.tile as tile
    from concourse import bacc
    from concourse._compat import get_trn_type
    from concourse.tile_rust import add_dep_helper

    f32 = mybir.dt.float32
    bf16 = mybir.dt.bfloat16

    nc = bacc.Bacc(get_trn_type() or "TRN2", target_bir_lowering=False, debug=False)
    q_h = nc.dram_tensor("q", [S, D], f32, kind="ExternalInput")
    k_h = nc.dram_tensor("k", [S, D], f32, kind="ExternalInput")
    v_h = nc.dram_tensor("v", [S, D], f32, kind="ExternalInput")
    Wq_h = nc.dram_tensor("Wq", [D, D], f32, kind="ExternalInput")
    bq_h = nc.dram_tensor("bq", [D], f32, kind="ExternalInput")
    Wk_h = nc.dram_tensor("Wk", [D, D], f32, kind="ExternalInput")
    bk_h = nc.dram_tensor("bk", [D], f32, kind="ExternalInput")
    Wv_h = nc.dram_tensor("Wv", [D, D], f32, kind="ExternalInput")
    bv_h = nc.dram_tensor("bv", [D], f32, kind="ExternalInput")
    out_h = nc.dram_tensor("out", [S, D], f32, kind="ExternalOutput")

    with tile.TileContext(nc) as tc:
        with (
            tc.tile_pool(name="consts", bufs=1) as consts,
            tc.tile_pool(name="ld", bufs=2) as ld,
            tc.tile_pool(name="nat", bufs=3) as nat,
            tc.tile_pool(name="big", bufs=1) as big,
            tc.tile_pool(name="expp", bufs=3) as expp,
            tc.tile_pool(name="outp", bufs=3) as outp,
            tc.tile_pool(name="small", bufs=4) as small,
            tc.tile_pool(name="ps_mm", bufs=3, space="PSUM") as ps_mm,
            tc.tile_pool(name="ps_out", bufs=2, space="PSUM") as ps_out,
            tc.tile_pool(name="ps_rs", bufs=2, space="PSUM") as ps_rs,
        ):
            chain = [None]

            def chained(ins):
                if chain[0] is not None:
                    add_dep_helper(ins.ins, chain[0].ins, sync=False,
                                   reason="wire order")
                chain[0] = ins
                return ins

            def load_W(W_h, i):
                w_sb = consts.tile([P, HT, D], bf16, tag=f"w{i}")
                wf = ld.tile([P, HT, D], f32, tag="ld")
                chained(nc.sync.dma_start(
                    out=wf, in_=W_h[:].rearrange("(c p) j -> p c j", p=P)))
                nc.vector.tensor_copy(out=w_sb, in_=wf)
                return w_sb

            # one input chunk: 1 MB load + DVE cast + 4x 4-in-1 xbar
            # transposes (out [p, hi, j] holds logical row hi*128+p)
            def load_chunk(x_h, xT, sc):
                xf = ld.tile([P, HT, D], f32, tag="ld")
                chained(nc.sync.dma_start(
                    out=xf,
                    in_=x_h[sc * QC:(sc + 1) * QC, :].rearrange(
                        "(c p) j -> p c j", p=P),
                ))
                xb = nat.tile([P, HT, D], bf16, tag="nat")
                for i in range(HT):
                    st = sc * HT + i
                    nc.vector.tensor_copy(out=xb[:, i, :], in_=xf[:, i, :])
                    chained(nc.sync.dma_start_transpose(
                        out=xT[:, :, st * P:(st + 1) * P], in_=xb[:, i, :]))

            for _rep in range(repeat):
                # small constants (SWDGE; tiny and ahead of all transposes)
                bq_sb = consts.tile([P, HT], f32, tag="bq")
                nc.gpsimd.dma_start(
                    out=bq_sb, in_=bq_h[:].rearrange("(di p) -> p di", p=P))
                bk_sb = consts.tile([P, HT], f32, tag="bk")
                nc.gpsimd.dma_start(
                    out=bk_sb, in_=bk_h[:].rearrange("(di p) -> p di", p=P))
                bv_bcast = consts.tile([P, D], f32, tag="bv")
                nc.gpsimd.dma_start(
                    out=bv_bcast,
                    in_=bass.AP(tensor=bv_h[:].tensor, offset=0,
                                ap=[[0, P], [1, D]]),
                )
                ones_bf = consts.tile([P, P], bf16, tag="ones")
                nc.vector.memset(ones_bf, 1.0)

                kT = big.tile([P, HT, S], bf16, tag="kT")
                qT = big.tile([P, HT, S], bf16, tag="qT")
                vT = big.tile([P, HT, S], bf16, tag="vT")
                kpT = big.tile([P, HT, S], bf16, tag="kpT")
                qpT = big.tile([P, HT, S], bf16, tag="qpT")
                vp = big.tile([P, ST, D], bf16, tag="vp")

                # wire order: wk k0 k1 wq q0 k2 k3 q1 wv v0 v1 q2 v2 v3 q3
                wk = load_W(Wk_h, 1)
                load_chunk(k_h, kT, 0)
                load_chunk(k_h, kT, 1)
                wq = load_W(Wq_h, 0)
                load_chunk(q_h, qT, 0)
                load_chunk(k_h, kT, 2)
                load_chunk(k_h, kT, 3)
                load_chunk(q_h, qT, 1)
                wv = load_W(Wv_h, 2)
                load_chunk(v_h, vT, 0)
                load_chunk(v_h, vT, 1)
                load_chunk(q_h, qT, 2)
                load_chunk(v_h, vT, 2)
                load_chunk(v_h, vT, 3)
                load_chunk(q_h, qT, 3)

                def project_chunk(xT, w_sb, b_sb, xpT, sc):
                    for di in range(HT):
                        ps = ps_mm.tile([P, QC], f32, tag="ps_mm")
                        for hi in range(HT):
                            nc.tensor.matmul(
                                ps,
                                w_sb[:, hi, di * P:(di + 1) * P],
                                xT[:, hi, sc * QC:(sc + 1) * QC],
                                start=(hi == 0),
                                stop=(hi == HT - 1),
                            )
                        nc.vector.tensor_scalar_add(
                            out=xpT[:, di, sc * QC:(sc + 1) * QC],
                            in0=ps,
                            scalar1=b_sb[:, di:di + 1],
                        )

                def vp_chunk(sc):
                    for st in range(sc * HT, (sc + 1) * HT):
                        ps = ps_mm.tile([P, D], f32, tag="ps_mm")
                        for hi in range(HT):
                            nc.tensor.matmul(
                                ps,
                                vT[:, hi, st * P:(st + 1) * P],
                                wv[:, hi, :],
                                start=(hi == 0),
                                stop=(hi == HT - 1),
                            )
                        nc.vector.tensor_add(vp[:, st, :], ps, bv_bcast)

                # scores block: kt tiles kc*4..kc*4+3 of q-chunk qc, with exp
                def scores_block(qc, kc, ex):
                    for kt in range(kc * HT, (kc + 1) * HT):
                        ps = ps_mm.tile([P, QC], f32, tag="ps_mm")
                        for di in range(HT):
                            nc.tensor.matmul(
                                ps,
                                kpT[:, di, kt * P:(kt + 1) * P],
                                qpT[:, di, qc * QC:(qc + 1) * QC],
                                start=(di == 0),
                                stop=(di == HT - 1),
                            )
                        nc.scalar.activation(
                            out=ex[:, kt, :],
                            in_=ps,
                            func=mybir.ActivationFunctionType.Exp,
                            scale=float(SCALE),
                        )

                def attn_chunk(qc, ex):
                    for qi in range(QC // P):
                        ps_o = ps_out.tile([P, D], f32, tag="ps_out")
                        ps_r = ps_rs.tile([P, 1], f32, tag="ps_rs")
                        for kt in range(ST):
                            lt = ex[:, kt, qi * P:(qi + 1) * P]
                            nc.tensor.matmul(
                                ps_o, lt, vp[:, kt, :],
                                start=(kt == 0), stop=(kt == ST - 1),
                            )
                            nc.tensor.matmul(
                                ps_r, lt, ones_bf[:, :1],
                                start=(kt == 0), stop=(kt == ST - 1),
                            )
                        rc = small.tile([P, 1], f32, tag="rc")
                        nc.vector.reciprocal(rc, ps_r)
                        ob = outp.tile([P, D], f32, tag="ob")
                        nc.vector.tensor_scalar_mul(ob, ps_o, rc)
                        qt = qc * (QC // P) + qi
                        nc.gpsimd.dma_start(
                            out=out_h[qt * P:(qt + 1) * P, :], in_=ob)

                # PE wavefront
                exs = []
                project_chunk(kT, wk, bk_sb, kpT, 0)
                project_chunk(kT, wk, bk_sb, kpT, 1)
                project_chunk(qT, wq, bq_sb, qpT, 0)
                ex0 = expp.tile([P, ST, QC], bf16, tag="ex")
                exs.append(ex0)
                scores_block(0, 0, ex0)
                scores_block(0, 1, ex0)
                project_chunk(kT, wk, bk_sb, kpT, 2)
                scores_block(0, 2, ex0)
                project_chunk(kT, wk, bk_sb, kpT, 3)
                scores_block(0, 3, ex0)
                ex1 = expp.tile([P, ST, QC], bf16, tag="ex")
                exs.append(ex1)
                project_chunk(qT, wq, bq_sb, qpT, 1)
                for kc in range(NQC):
                    scores_block(1, kc, ex1)
                for sc in range(NQC):
                    vp_chunk(sc)
                attn_chunk(0, exs[0])
                ex2 = expp.tile([P, ST, QC], bf16, tag="ex")
                exs.append(ex2)
                project_chunk(qT, wq, bq_sb, qpT, 2)
                for kc in range(NQC):
                    scores_block(2, kc, ex2)
                attn_chunk(1, exs[1])
                ex3 = expp.tile([P, ST, QC], bf16, tag="ex")
                exs.append(ex3)
                project_chunk(qT, wq, bq_sb, qpT, 3)
                for kc in range(NQC):
                    scores_block(3, kc, ex3)
                attn_chunk(2, exs[2])
                attn_chunk(3, exs[3])

    nc.compile()
    return nc


def _get_nc():
    global _NC
    if _NC is None:
        _NC = build_nc()
    return _NC


def build_in_maps(q, k, v, Wq, bq, Wk, bk, Wv, bv):
    in_maps = []
    for b in range(B):
        in_maps.append({
            "q": np.ascontiguousarray(q[b], dtype=np.float32),
            "k": np.ascontiguousarray(k[b], dtype=np.float32),
            "v": np.ascontiguousarray(v[b], dtype=np.float32),
            "Wq": np.ascontiguousarray(Wq, dtype=np.float32),
            "bq": np.ascontiguousarray(bq, dtype=np.float32),
            "Wk": np.ascontiguousarray(Wk, dtype=np.float32),
            "bk": np.ascontiguousarray(bk, dtype=np.float32),
            "Wv": np.ascontiguousarray(Wv, dtype=np.float32),
            "bv": np.ascontiguousarray(bv, dtype=np.float32),
        })
    return in_maps


def kernel(q, k, v, Wq, bq, Wk, bk, Wv, bv):
    from concourse.bass_utils import run_bass_kernel_spmd

    nc = _get_nc()
    in_maps = build_in_maps(q, k, v, Wq, bq, Wk, bk, Wv, bv)
    res = run_bass_kernel_spmd(nc, in_maps, core_ids=list(range(B)))
    return np.stack([r["out"] for r in res.results], axis=0)
